# revision 56
# baseline (speedup 1.0000x reference)
"""HGATConv (4-head graph attention, N=4096, F=512) on 8 Trainium2 NeuronCores.

Sharding: (head, node-half) grid — core c handles head c//2 and output rows
q*2048..(q+1)*2048 (q = c%2). Every core computes its head's h = x @ W_h for
ALL nodes locally (bf16 matmuls) — no collective at all. The host rotates
the node axis per core so each core's own nodes come first (attention sums
are j-order invariant); that makes the per-core x layout identical SPMD-wise.

Attention math per 128-node j-block (tiles are [128 j, 2048 i]): all scores
are divided by exp(s1_i), which is constant per softmax row and cancels in
the normalization:
  p'' = exp(leakyrelu(s1_i + s2_j)) / exp(s1_i)
      = max(exp(-0.8 s1_i + 0.2 s2_j), exp(s2_j))
      = max(E8_i * g_j, e1_j)          (separable rank-1 first branch!)
with E8_i = exp(-0.8 s1_i) a jb-invariant broadcast tile, g_j = exp(0.2 s2_j)
and e1_j = exp(s2_j) per-partition scalars. So the whole score+leakyrelu is
ONE 4x-rate two-scalar tensor_scalar per block — no wide ACT exp at all; the
only wide 2x op left is the adjacency-mask multiply.
Row sums ride along as a ones-column appended to h (129-wide matmuls).

x arrives in four k-interleaved quarter DMAs so the h-compute starts after
the first MiB; the h-compute loop is emitted interleaved with the attention
loop (lag 4) and PSUM accumulators are packed 3-per-bank so both phases'
PSUM pools coexist and all engines pipeline across the phases.
"""

import sys
import numpy as np

if "/opt/trn_rl_repo" not in sys.path:
    sys.path.insert(0, "/opt/trn_rl_repo")

H, D = 4, 128          # heads, head dim
N, F = 4096, 512       # nodes, features
M = 8                  # cores
NOWN = 1024 * 2        # 2048 own output rows per core
JB = N // 128          # 32 j blocks
IB = NOWN // 128       # 16 own-row blocks
KB = F // 128          # 4 contraction blocks
DA = D + 2             # head W columns + wa2 + wa1
NQ = N // 4            # nodes per x-quarter DMA
LAG = 4                # h-compute blocks emitted ahead of attention blocks

_CACHE = {}

# attention-matmul emission order: s=0 slices first (their start=True clears
# the bank), consecutive matmuls on different PSUM banks
_MM_ORDER = [0, 3, 6, 9, 12, 15, 1, 4, 7, 10, 13, 2, 5, 8, 11, 14]


def _build_nc():
    import concourse.bacc as bacc
    from concourse import mybir
    from concourse.tile import TileContext

    f32 = mybir.dt.float32
    bf16 = mybir.dt.bfloat16
    Alu = mybir.AluOpType
    Act = mybir.ActivationFunctionType

    nc = bacc.Bacc()
    # x, k-interleaved and quarter-major: one DMA slice brings every
    # contraction block for a 1024-node range (device node order is the
    # per-core rotated order; own nodes are quarters 0-1)
    xc_d = nc.declare_dram_parameter("xc", [128, KB * N], bf16, isOutput=False)
    Wh_d = nc.declare_dram_parameter("Wh", [128, KB * DA], bf16, isOutput=False)
    maskT_d = nc.declare_dram_parameter("maskT", [N, NOWN], bf16, isOutput=False)
    # wa1 replicated across 128 columns: E8b comes from one matmul layer
    wa1r_d = nc.declare_dram_parameter("wa1r", [128, KB * 128], bf16,
                                       isOutput=False)
    out_d = nc.declare_dram_parameter("out", [NOWN, D], f32, isOutput=True)

    with TileContext(nc) as tc:
        with tc.tile_pool(name="const", bufs=1) as const_pool:
            Wh_sb = const_pool.tile([128, KB * DA], bf16)
            nc.sync.dma_start(Wh_sb[:], Wh_d[:])
            wa1r = const_pool.tile([128, KB * 128], bf16)
            nc.sync.dma_start(wa1r[:], wa1r_d[:])
            xparts = [const_pool.tile([128, KB * NQ], bf16, name=f"xp{qt}")
                      for qt in range(4)]
            nc.sync.dma_start(xparts[0][:], xc_d[:, 0:KB * NQ])
            nc.sync.dma_start(xparts[1][:], xc_d[:, KB * NQ:2 * KB * NQ])

            def xkb(k, b):
                """x block [128 k-rows, 128 nodes] for contraction k, node block b."""
                qt, bq = divmod(b, 8)
                off = k * NQ + bq * 128
                return xparts[qt][:, off:off + 128]

            haug = const_pool.tile([128, JB * (D + 1)], bf16)   # [h | 1] per block
            e1all = const_pool.tile([128, JB], f32)             # exp(s2) per block
            gall = const_pool.tile([128, JB], f32)              # exp(0.2 s2) per block
            E8b = const_pool.tile([128, NOWN], bf16)            # exp(-0.8 s1) bcast
            # separate quarter copies of E8b: the first attention blocks run
            # chunked against these so they start before all of E8b is ready
            E8q = [const_pool.tile([128, 512], bf16, name=f"E8q{c}")
                   for c in range(4)]

            # ones columns for the row-sum ride-along, one strided memset
            haug3 = haug.rearrange("p (b c) -> p b c", c=D + 1)
            nc.vector.memset(haug3[:, :, D:D + 1], 1.0)

            with (
                tc.tile_pool(name="stream", bufs=5) as stream,
                tc.tile_pool(name="tail", bufs=1) as tail_pool,
            ):
                def emit_h_block(b, pool):
                    ph = pool.tile([128, D + 1], f32, tag="ph")
                    for k in range(KB):
                        nc.tensor.matmul(
                            ph[:],
                            lhsT=xkb(k, b),
                            rhs=Wh_sb[:, k * DA:k * DA + D + 1],
                            start=(k == 0), stop=(k == KB - 1))
                    nc.scalar.activation(haug[:, b * (D + 1):b * (D + 1) + D],
                                         ph[:, 0:D], Act.Copy)
                    nc.scalar.activation(e1all[:, b:b + 1], ph[:, D:D + 1],
                                         Act.Exp)
                    nc.scalar.activation(gall[:, b:b + 1], ph[:, D:D + 1],
                                         Act.Exp, scale=0.2)

                with tc.tile_pool(name="pre", bufs=2, space="PSUM") as pre:
                    def emit_e8b(c4):
                        # pb[p, i] = s1_i for own cols c4*512.. directly from
                        # x via the replicated-wa1 stationary, then exp
                        pb = pre.tile([128, 512], f32, tag="pb")
                        qt, half = divmod(c4, 2)
                        for k in range(KB):
                            nc.tensor.matmul(
                                pb[:],
                                lhsT=wa1r[:, k * 128:(k + 1) * 128],
                                rhs=xparts[qt][:, k * NQ + half * 512:
                                               k * NQ + (half + 1) * 512],
                                start=(k == 0), stop=(k == KB - 1))
                        nc.scalar.activation(E8q[c4][:], pb[:],
                                             Act.Exp, scale=-0.8)
                        nc.scalar.activation(E8b[:, c4 * 512:(c4 + 1) * 512],
                                             pb[:], Act.Exp, scale=-0.8)

                    for c4 in range(4):
                        emit_e8b(c4)
                    for b in range(4):
                        emit_h_block(b, pre)

                with (
                    tc.tile_pool(name="acc", bufs=1, space="PSUM") as acc_pool,
                    tc.tile_pool(name="hpsum", bufs=2, space="PSUM") as hpsum,
                ):
                    # 5 tiles x 3 slices + 1 tile x 1 slice = 16 accumulators
                    acc = [acc_pool.tile([128, 3 * (D + 1)], f32,
                                         name=f"acc{t}") for t in range(5)]
                    acc.append(acc_pool.tile([128, D + 1], f32, name="acc5"))

                    def emit_attn_block(jb):
                        mask = stream.tile([128, NOWN], bf16, tag="mask")
                        nc.sync.dma_start(mask[:],
                                          maskT_d[jb * 128:(jb + 1) * 128, :])
                        if jb < 2:
                            # chunked against the E8b quarter tiles so DVE
                            # starts before the whole prelude finishes
                            pmc = []
                            for c in range(4):
                                sl = slice(c * 512, (c + 1) * 512)
                                q1c = stream.tile([128, 512], bf16,
                                                  tag=f"q1c{c}")
                                nc.vector.tensor_scalar(
                                    q1c[:], in0=E8q[c][:],
                                    scalar1=gall[:, jb:jb + 1],
                                    scalar2=e1all[:, jb:jb + 1],
                                    op0=Alu.mult, op1=Alu.max)
                                pc = stream.tile([128, 512], bf16,
                                                 tag=f"pmc{c}")
                                nc.vector.tensor_tensor(pc[:], q1c[:],
                                                        mask[:, sl],
                                                        op=Alu.mult)
                                pmc.append(pc)

                            def pm_slice(ib):
                                c, r = divmod(ib, 4)
                                return pmc[c][:, r * 128:(r + 1) * 128]
                        else:
                            # q1 = max(E8_i * g_j, e1_j) in one two-scalar TS
                            q1 = stream.tile([128, NOWN], bf16, tag="q1")
                            nc.vector.tensor_scalar(q1[:], in0=E8b[:],
                                                    scalar1=gall[:, jb:jb + 1],
                                                    scalar2=e1all[:, jb:jb + 1],
                                                    op0=Alu.mult, op1=Alu.max)
                            pm = stream.tile([128, NOWN], bf16, tag="pm")
                            nc.vector.tensor_tensor(pm[:], q1[:], mask[:],
                                                    op=Alu.mult)

                            def pm_slice(ib):
                                return pm[:, ib * 128:(ib + 1) * 128]

                        for ib in _MM_ORDER:
                            t8, s8 = divmod(ib, 3)
                            last = (s8 == 2) or (t8 == 5)
                            nc.tensor.matmul(
                                acc[t8][:, s8 * (D + 1):(s8 + 1) * (D + 1)],
                                lhsT=pm_slice(ib),
                                rhs=haug[:, jb * (D + 1):(jb + 1) * (D + 1)],
                                start=(jb == 0 and s8 == 0),
                                stop=(jb == JB - 1 and last),
                                skip_group_check=True)

                    for step in range(4, JB + LAG):
                        if step == 6:
                            nc.sync.dma_start(xparts[2][:],
                                              xc_d[:, 2 * KB * NQ:3 * KB * NQ])
                        if step == 12:
                            nc.sync.dma_start(xparts[3][:],
                                              xc_d[:, 3 * KB * NQ:4 * KB * NQ])
                        if step < JB:
                            emit_h_block(step, hpsum)
                        if step >= LAG:
                            emit_attn_block(step - LAG)

                    # ---- tail: normalize + elu + store, two fully pipelined
                    # halves (recip -> norm -> elu -> DMA per half) ----
                    osb = tail_pool.tile([128, NOWN], f32, tag="osb")
                    rinv = tail_pool.tile([128, IB], f32, tag="rinv")
                    zmin = tail_pool.tile([128, NOWN], f32, tag="zmin")
                    ez = tail_pool.tile([128, NOWN], f32, tag="ez")
                    rm1 = tail_pool.tile([128, NOWN], f32, tag="rm1")
                    oo = tail_pool.tile([128, NOWN], f32, tag="oo")
                    out3 = out_d.rearrange("(b p) d -> p b d", p=128)
                    oo3 = oo.rearrange("p (b d) -> p b d", d=D)
                    HN = NOWN // 2
                    for hf in range(2):
                        for ib in range(hf * 8, hf * 8 + 8):
                            t8, s = divmod(ib, 3)
                            nc.vector.reciprocal(
                                rinv[:, ib:ib + 1],
                                acc[t8][:, s * (D + 1) + D:
                                        s * (D + 1) + D + 1])
                        for ib in range(hf * 8, hf * 8 + 8):
                            t8, s = divmod(ib, 3)
                            nc.scalar.activation(
                                osb[:, ib * 128:(ib + 1) * 128],
                                acc[t8][:, s * (D + 1):s * (D + 1) + D],
                                Act.Copy, scale=rinv[:, ib:ib + 1])
                        # elu(x) = (relu(x) - 1) + exp(min(x, 0))
                        sl = slice(hf * HN, (hf + 1) * HN)
                        nc.vector.tensor_scalar(zmin[:, sl], in0=osb[:, sl],
                                                scalar1=0.0, scalar2=None,
                                                op0=Alu.min)
                        nc.scalar.activation(ez[:, sl], zmin[:, sl], Act.Exp)
                        nc.vector.tensor_scalar(rm1[:, sl], in0=osb[:, sl],
                                                scalar1=0.0, scalar2=-1.0,
                                                op0=Alu.max, op1=Alu.add)
                        nc.vector.tensor_tensor(oo[:, sl], ez[:, sl],
                                                rm1[:, sl], op=Alu.add)
                        bs = slice(hf * (IB // 2), (hf + 1) * (IB // 2))
                        nc.sync.dma_start(out3[:, bs, :], oo3[:, bs, :])

    nc.compile()
    return nc


def _host_prep(x, adj, W, a):
    import ml_dtypes
    x = np.asarray(x, np.float32)
    adj = np.asarray(adj)
    W = np.asarray(W, np.float32)
    a = np.asarray(a, np.float32)

    xT = x.T.astype(ml_dtypes.bfloat16)                                # [F, N]
    adjT = adj.T.astype(ml_dtypes.bfloat16)                            # [j, i]

    in_maps = []
    for c in range(M):
        hd, q = divmod(c, 2)
        Wh = W[:, hd * D:(hd + 1) * D]                                  # [F, D]
        wa1 = Wh @ a[:D, 0]
        wa2 = Wh @ a[D:, 0]
        # [W_h | wa2 | wa1]: col D = s2 weights, col D+1 = s1 weights;
        # k-interleaved to load with one DMA
        Whc = np.concatenate([Wh, wa2[:, None], wa1[:, None]], axis=1)
        Whc = np.ascontiguousarray(
            Whc.astype(ml_dtypes.bfloat16).reshape(KB, 128, DA)
            .transpose(1, 0, 2).reshape(128, KB * DA))
        # wa1 replicated across 128 stationary columns, k-interleaved
        wa1r = np.ascontiguousarray(
            np.broadcast_to(
                wa1.astype(ml_dtypes.bfloat16).reshape(KB, 128, 1),
                (KB, 128, 128)).transpose(1, 0, 2).reshape(128, KB * 128))
        # rotate the node axis so own nodes are first, then k/quarter-interleave
        xrot = np.concatenate([xT[:, q * NOWN:], xT[:, :q * NOWN]], axis=1)
        xc = np.ascontiguousarray(
            xrot.reshape(KB, 128, 4, NQ).transpose(2, 1, 0, 3).reshape(4, 128, -1)
            .transpose(1, 0, 2).reshape(128, -1))
        adjrot = np.concatenate([adjT[q * NOWN:], adjT[:q * NOWN]], axis=0)
        in_maps.append({
            "xc": xc,
            "Wh": Whc,
            "maskT": np.ascontiguousarray(adjrot[:, q * NOWN:(q + 1) * NOWN]),
            "wa1r": wa1r,
        })
    return in_maps


def kernel(x, adj, W, a):
    from concourse.bass_utils import run_bass_kernel_spmd

    if "nc" not in _CACHE:
        _CACHE["nc"] = _build_nc()
    nc = _CACHE["nc"]

    in_maps = _host_prep(x, adj, W, a)
    res = run_bass_kernel_spmd(nc, in_maps, list(range(M)))
    out = np.empty((N, H * D), np.float32)
    for c in range(M):
        hd, q = divmod(c, 2)
        out[q * NOWN:(q + 1) * NOWN, hd * D:(hd + 1) * D] = np.asarray(
            res.results[c]["out"], np.float32)
    return out


if __name__ == "__main__":
    nc = _build_nc()
    print("built ok")


# revision 57
# speedup vs baseline: 1.0180x; 1.0180x over previous
"""HGATConv (4-head graph attention, N=4096, F=512) on 8 Trainium2 NeuronCores.

Sharding: (head, node-half) grid — core c handles head c//2 and output rows
q*2048..(q+1)*2048 (q = c%2). Every core computes its head's h = x @ W_h for
ALL nodes locally (bf16 matmuls) — no collective at all. The host rotates
the node axis per core so each core's own nodes come first (attention sums
are j-order invariant); that makes the per-core x layout identical SPMD-wise.

Attention math per 128-node j-block (tiles are [128 j, 2048 i]): all scores
are divided by exp(s1_i), which is constant per softmax row and cancels in
the normalization:
  p'' = exp(leakyrelu(s1_i + s2_j)) / exp(s1_i)
      = max(exp(-0.8 s1_i + 0.2 s2_j), exp(s2_j))
      = max(E8_i * g_j, e1_j)          (separable rank-1 first branch!)
with E8_i = exp(-0.8 s1_i) a jb-invariant broadcast tile, g_j = exp(0.2 s2_j)
and e1_j = exp(s2_j) per-partition scalars. So the whole score+leakyrelu is
ONE 4x-rate two-scalar tensor_scalar per block — no wide ACT exp at all; the
only wide 2x op left is the adjacency-mask multiply.
Row sums ride along as a ones-column appended to h (129-wide matmuls).

x arrives in four k-interleaved quarter DMAs so the h-compute starts after
the first MiB; the h-compute loop is emitted interleaved with the attention
loop (lag 4) and PSUM accumulators are packed 3-per-bank so both phases'
PSUM pools coexist and all engines pipeline across the phases.
"""

import sys
import numpy as np

if "/opt/trn_rl_repo" not in sys.path:
    sys.path.insert(0, "/opt/trn_rl_repo")

H, D = 4, 128          # heads, head dim
N, F = 4096, 512       # nodes, features
M = 8                  # cores
NOWN = 1024 * 2        # 2048 own output rows per core
JB = N // 128          # 32 j blocks
IB = NOWN // 128       # 16 own-row blocks
KB = F // 128          # 4 contraction blocks
DA = D + 2             # head W columns + wa2 + wa1
NQ = N // 4            # nodes per x-quarter DMA
LAG = 4                # h-compute blocks emitted ahead of attention blocks

_CACHE = {}

# attention-matmul emission order: s=0 slices first (their start=True clears
# the bank), consecutive matmuls on different PSUM banks
_MM_ORDER = [0, 3, 6, 9, 12, 15, 1, 4, 7, 10, 13, 2, 5, 8, 11, 14]


def _build_nc():
    import concourse.bacc as bacc
    from concourse import mybir
    from concourse.tile import TileContext

    f32 = mybir.dt.float32
    bf16 = mybir.dt.bfloat16
    Alu = mybir.AluOpType
    Act = mybir.ActivationFunctionType

    nc = bacc.Bacc()
    # x, k-interleaved and quarter-major: one DMA slice brings every
    # contraction block for a 1024-node range (device node order is the
    # per-core rotated order; own nodes are quarters 0-1)
    xc_d = nc.declare_dram_parameter("xc", [128, KB * N], bf16, isOutput=False)
    Wh_d = nc.declare_dram_parameter("Wh", [128, KB * DA], bf16, isOutput=False)
    maskT_d = nc.declare_dram_parameter("maskT", [N, NOWN], bf16, isOutput=False)
    # wa1 replicated across 128 columns: E8b comes from one matmul layer
    wa1r_d = nc.declare_dram_parameter("wa1r", [128, KB * 128], bf16,
                                       isOutput=False)
    out_d = nc.declare_dram_parameter("out", [NOWN, D], f32, isOutput=True)

    with TileContext(nc) as tc:
        with tc.tile_pool(name="const", bufs=1) as const_pool:
            Wh_sb = const_pool.tile([128, KB * DA], bf16)
            nc.sync.dma_start(Wh_sb[:], Wh_d[:])
            wa1r = const_pool.tile([128, KB * 128], bf16)
            nc.sync.dma_start(wa1r[:], wa1r_d[:])
            xparts = [const_pool.tile([128, KB * NQ], bf16, name=f"xp{qt}")
                      for qt in range(4)]
            nc.sync.dma_start(xparts[0][:], xc_d[:, 0:KB * NQ])
            nc.sync.dma_start(xparts[1][:], xc_d[:, KB * NQ:2 * KB * NQ])

            def xkb(k, b):
                """x block [128 k-rows, 128 nodes] for contraction k, node block b."""
                qt, bq = divmod(b, 8)
                off = k * NQ + bq * 128
                return xparts[qt][:, off:off + 128]

            haug = const_pool.tile([128, JB * (D + 1)], bf16)   # [h | 1] per block
            e1all = const_pool.tile([128, JB], f32)             # exp(s2) per block
            gall = const_pool.tile([128, JB], f32)              # exp(0.2 s2) per block
            E8b = const_pool.tile([128, NOWN], bf16)            # exp(-0.8 s1) bcast
            # separate quarter copies of E8b: the first attention blocks run
            # chunked against these so they start before all of E8b is ready
            E8q = [const_pool.tile([128, 512], bf16, name=f"E8q{c}")
                   for c in range(4)]

            # ones columns for the row-sum ride-along, one strided memset
            haug3 = haug.rearrange("p (b c) -> p b c", c=D + 1)
            nc.vector.memset(haug3[:, :, D:D + 1], 1.0)

            with (
                tc.tile_pool(name="stream", bufs=5) as stream,
                tc.tile_pool(name="tail", bufs=1) as tail_pool,
            ):
                def emit_h_block(b, pool):
                    ph = pool.tile([128, D + 1], f32, tag="ph")
                    for k in range(KB):
                        nc.tensor.matmul(
                            ph[:],
                            lhsT=xkb(k, b),
                            rhs=Wh_sb[:, k * DA:k * DA + D + 1],
                            start=(k == 0), stop=(k == KB - 1))
                    nc.scalar.activation(haug[:, b * (D + 1):b * (D + 1) + D],
                                         ph[:, 0:D], Act.Copy)
                    nc.scalar.activation(e1all[:, b:b + 1], ph[:, D:D + 1],
                                         Act.Exp)
                    nc.scalar.activation(gall[:, b:b + 1], ph[:, D:D + 1],
                                         Act.Exp, scale=0.2)

                with tc.tile_pool(name="pre", bufs=2, space="PSUM") as pre:
                    def emit_e8b(c4):
                        # pb[p, i] = s1_i for own cols c4*512.. directly from
                        # x via the replicated-wa1 stationary, then exp
                        pb = pre.tile([128, 512], f32, tag="pb")
                        qt, half = divmod(c4, 2)
                        for k in range(KB):
                            nc.tensor.matmul(
                                pb[:],
                                lhsT=wa1r[:, k * 128:(k + 1) * 128],
                                rhs=xparts[qt][:, k * NQ + half * 512:
                                               k * NQ + (half + 1) * 512],
                                start=(k == 0), stop=(k == KB - 1))
                        nc.scalar.activation(E8q[c4][:], pb[:],
                                             Act.Exp, scale=-0.8)
                        nc.scalar.activation(E8b[:, c4 * 512:(c4 + 1) * 512],
                                             pb[:], Act.Exp, scale=-0.8)

                    # interleave so h-block 0 (whose scores gate the first
                    # attention block) isn't queued behind all E8b matmuls
                    for c4 in range(4):
                        emit_e8b(c4)
                        emit_h_block(c4, pre)

                with (
                    tc.tile_pool(name="acc", bufs=1, space="PSUM") as acc_pool,
                    tc.tile_pool(name="hpsum", bufs=2, space="PSUM") as hpsum,
                ):
                    # 5 tiles x 3 slices + 1 tile x 1 slice = 16 accumulators
                    acc = [acc_pool.tile([128, 3 * (D + 1)], f32,
                                         name=f"acc{t}") for t in range(5)]
                    acc.append(acc_pool.tile([128, D + 1], f32, name="acc5"))

                    def emit_attn_block(jb):
                        mask = stream.tile([128, NOWN], bf16, tag="mask")
                        nc.sync.dma_start(mask[:],
                                          maskT_d[jb * 128:(jb + 1) * 128, :])
                        if jb < 2:
                            # chunked against the E8b quarter tiles so DVE
                            # starts before the whole prelude finishes
                            pmc = []
                            for c in range(4):
                                sl = slice(c * 512, (c + 1) * 512)
                                q1c = stream.tile([128, 512], bf16,
                                                  tag=f"q1c{c}")
                                nc.vector.tensor_scalar(
                                    q1c[:], in0=E8q[c][:],
                                    scalar1=gall[:, jb:jb + 1],
                                    scalar2=e1all[:, jb:jb + 1],
                                    op0=Alu.mult, op1=Alu.max)
                                pc = stream.tile([128, 512], bf16,
                                                 tag=f"pmc{c}")
                                nc.vector.tensor_tensor(pc[:], q1c[:],
                                                        mask[:, sl],
                                                        op=Alu.mult)
                                pmc.append(pc)

                            def pm_slice(ib):
                                c, r = divmod(ib, 4)
                                return pmc[c][:, r * 128:(r + 1) * 128]
                        else:
                            # q1 = max(E8_i * g_j, e1_j) in one two-scalar TS
                            q1 = stream.tile([128, NOWN], bf16, tag="q1")
                            nc.vector.tensor_scalar(q1[:], in0=E8b[:],
                                                    scalar1=gall[:, jb:jb + 1],
                                                    scalar2=e1all[:, jb:jb + 1],
                                                    op0=Alu.mult, op1=Alu.max)
                            pm = stream.tile([128, NOWN], bf16, tag="pm")
                            nc.vector.tensor_tensor(pm[:], q1[:], mask[:],
                                                    op=Alu.mult)

                            def pm_slice(ib):
                                return pm[:, ib * 128:(ib + 1) * 128]

                        for ib in _MM_ORDER:
                            t8, s8 = divmod(ib, 3)
                            last = (s8 == 2) or (t8 == 5)
                            nc.tensor.matmul(
                                acc[t8][:, s8 * (D + 1):(s8 + 1) * (D + 1)],
                                lhsT=pm_slice(ib),
                                rhs=haug[:, jb * (D + 1):(jb + 1) * (D + 1)],
                                start=(jb == 0 and s8 == 0),
                                stop=(jb == JB - 1 and last),
                                skip_group_check=True)

                    for step in range(4, JB + LAG):
                        if step == 6:
                            nc.sync.dma_start(xparts[2][:],
                                              xc_d[:, 2 * KB * NQ:3 * KB * NQ])
                        if step == 12:
                            nc.sync.dma_start(xparts[3][:],
                                              xc_d[:, 3 * KB * NQ:4 * KB * NQ])
                        if step < JB:
                            emit_h_block(step, hpsum)
                        if step >= LAG:
                            emit_attn_block(step - LAG)

                    # ---- tail: normalize + elu + store, two fully pipelined
                    # halves (recip -> norm -> elu -> DMA per half) ----
                    osb = tail_pool.tile([128, NOWN], f32, tag="osb")
                    rinv = tail_pool.tile([128, IB], f32, tag="rinv")
                    zmin = tail_pool.tile([128, NOWN], f32, tag="zmin")
                    ez = tail_pool.tile([128, NOWN], f32, tag="ez")
                    rm1 = tail_pool.tile([128, NOWN], f32, tag="rm1")
                    oo = tail_pool.tile([128, NOWN], f32, tag="oo")
                    out3 = out_d.rearrange("(b p) d -> p b d", p=128)
                    oo3 = oo.rearrange("p (b d) -> p b d", d=D)
                    HN = NOWN // 2
                    for hf in range(2):
                        for ib in range(hf * 8, hf * 8 + 8):
                            t8, s = divmod(ib, 3)
                            nc.vector.reciprocal(
                                rinv[:, ib:ib + 1],
                                acc[t8][:, s * (D + 1) + D:
                                        s * (D + 1) + D + 1])
                        for ib in range(hf * 8, hf * 8 + 8):
                            t8, s = divmod(ib, 3)
                            nc.scalar.activation(
                                osb[:, ib * 128:(ib + 1) * 128],
                                acc[t8][:, s * (D + 1):s * (D + 1) + D],
                                Act.Copy, scale=rinv[:, ib:ib + 1])
                        # elu(x) = (relu(x) - 1) + exp(min(x, 0))
                        sl = slice(hf * HN, (hf + 1) * HN)
                        nc.vector.tensor_scalar(zmin[:, sl], in0=osb[:, sl],
                                                scalar1=0.0, scalar2=None,
                                                op0=Alu.min)
                        nc.scalar.activation(ez[:, sl], zmin[:, sl], Act.Exp)
                        nc.vector.tensor_scalar(rm1[:, sl], in0=osb[:, sl],
                                                scalar1=0.0, scalar2=-1.0,
                                                op0=Alu.max, op1=Alu.add)
                        nc.vector.tensor_tensor(oo[:, sl], ez[:, sl],
                                                rm1[:, sl], op=Alu.add)
                        bs = slice(hf * (IB // 2), (hf + 1) * (IB // 2))
                        nc.sync.dma_start(out3[:, bs, :], oo3[:, bs, :])

    nc.compile()
    return nc


def _host_prep(x, adj, W, a):
    import ml_dtypes
    x = np.asarray(x, np.float32)
    adj = np.asarray(adj)
    W = np.asarray(W, np.float32)
    a = np.asarray(a, np.float32)

    xT = x.T.astype(ml_dtypes.bfloat16)                                # [F, N]
    adjT = adj.T.astype(ml_dtypes.bfloat16)                            # [j, i]

    in_maps = []
    for c in range(M):
        hd, q = divmod(c, 2)
        Wh = W[:, hd * D:(hd + 1) * D]                                  # [F, D]
        wa1 = Wh @ a[:D, 0]
        wa2 = Wh @ a[D:, 0]
        # [W_h | wa2 | wa1]: col D = s2 weights, col D+1 = s1 weights;
        # k-interleaved to load with one DMA
        Whc = np.concatenate([Wh, wa2[:, None], wa1[:, None]], axis=1)
        Whc = np.ascontiguousarray(
            Whc.astype(ml_dtypes.bfloat16).reshape(KB, 128, DA)
            .transpose(1, 0, 2).reshape(128, KB * DA))
        # wa1 replicated across 128 stationary columns, k-interleaved
        wa1r = np.ascontiguousarray(
            np.broadcast_to(
                wa1.astype(ml_dtypes.bfloat16).reshape(KB, 128, 1),
                (KB, 128, 128)).transpose(1, 0, 2).reshape(128, KB * 128))
        # rotate the node axis so own nodes are first, then k/quarter-interleave
        xrot = np.concatenate([xT[:, q * NOWN:], xT[:, :q * NOWN]], axis=1)
        xc = np.ascontiguousarray(
            xrot.reshape(KB, 128, 4, NQ).transpose(2, 1, 0, 3).reshape(4, 128, -1)
            .transpose(1, 0, 2).reshape(128, -1))
        adjrot = np.concatenate([adjT[q * NOWN:], adjT[:q * NOWN]], axis=0)
        in_maps.append({
            "xc": xc,
            "Wh": Whc,
            "maskT": np.ascontiguousarray(adjrot[:, q * NOWN:(q + 1) * NOWN]),
            "wa1r": wa1r,
        })
    return in_maps


def kernel(x, adj, W, a):
    from concourse.bass_utils import run_bass_kernel_spmd

    if "nc" not in _CACHE:
        _CACHE["nc"] = _build_nc()
    nc = _CACHE["nc"]

    in_maps = _host_prep(x, adj, W, a)
    res = run_bass_kernel_spmd(nc, in_maps, list(range(M)))
    out = np.empty((N, H * D), np.float32)
    for c in range(M):
        hd, q = divmod(c, 2)
        out[q * NOWN:(q + 1) * NOWN, hd * D:(hd + 1) * D] = np.asarray(
            res.results[c]["out"], np.float32)
    return out


if __name__ == "__main__":
    nc = _build_nc()
    print("built ok")


# revision 60
# speedup vs baseline: 1.0343x; 1.0160x over previous
"""HGATConv (4-head graph attention, N=4096, F=512) on 8 Trainium2 NeuronCores.

Sharding: (head, node-half) grid — core c handles head c//2 and output rows
q*2048..(q+1)*2048 (q = c%2). Every core computes its head's h = x @ W_h for
ALL nodes locally (bf16 matmuls) — no collective at all. The host rotates
the node axis per core so each core's own nodes come first (attention sums
are j-order invariant); that makes the per-core x layout identical SPMD-wise.

Attention math per 128-node j-block (tiles are [128 j, 2048 i]): all scores
are divided by exp(s1_i), which is constant per softmax row and cancels in
the normalization:
  p'' = exp(leakyrelu(s1_i + s2_j)) / exp(s1_i)
      = max(exp(-0.8 s1_i + 0.2 s2_j), exp(s2_j))
      = max(E8_i * g_j, e1_j)          (separable rank-1 first branch!)
with E8_i = exp(-0.8 s1_i) a jb-invariant broadcast tile, g_j = exp(0.2 s2_j)
and e1_j = exp(s2_j) per-partition scalars. So the whole score+leakyrelu is
ONE 4x-rate two-scalar tensor_scalar per block — no wide ACT exp at all; the
only wide 2x op left is the adjacency-mask multiply.
Row sums ride along as a ones-column appended to h (129-wide matmuls).

x arrives in four k-interleaved quarter DMAs so the h-compute starts after
the first MiB; the h-compute loop is emitted interleaved with the attention
loop (lag 4) and PSUM accumulators are packed 3-per-bank so both phases'
PSUM pools coexist and all engines pipeline across the phases.
"""

import sys
import numpy as np

if "/opt/trn_rl_repo" not in sys.path:
    sys.path.insert(0, "/opt/trn_rl_repo")

H, D = 4, 128          # heads, head dim
N, F = 4096, 512       # nodes, features
M = 8                  # cores
NOWN = 1024 * 2        # 2048 own output rows per core
JB = N // 128          # 32 j blocks
IB = NOWN // 128       # 16 own-row blocks
KB = F // 128          # 4 contraction blocks
DA = D + 2             # head W columns + wa2 + wa1
NQ = N // 4            # nodes per x-quarter DMA
LAG = 4                # h-compute blocks emitted ahead of attention blocks

_CACHE = {}

# attention-matmul emission order: s=0 slices first (their start=True clears
# the bank), consecutive matmuls on different PSUM banks
_MM_ORDER = [0, 3, 6, 9, 12, 15, 1, 4, 7, 10, 13, 2, 5, 8, 11, 14]


def _build_nc():
    import concourse.bacc as bacc
    from concourse import mybir
    from concourse.tile import TileContext

    f32 = mybir.dt.float32
    bf16 = mybir.dt.bfloat16
    Alu = mybir.AluOpType
    Act = mybir.ActivationFunctionType

    nc = bacc.Bacc()
    # x, k-interleaved and quarter-major: one DMA slice brings every
    # contraction block for a 1024-node range (device node order is the
    # per-core rotated order; own nodes are quarters 0-1)
    xc_d = nc.declare_dram_parameter("xc", [128, KB * N], bf16, isOutput=False)
    Wh_d = nc.declare_dram_parameter("Wh", [128, KB * DA], bf16, isOutput=False)
    maskT_d = nc.declare_dram_parameter("maskT", [N, NOWN], bf16, isOutput=False)
    # wa1 replicated across 128 columns: E8b comes from one matmul layer
    wa1r_d = nc.declare_dram_parameter("wa1r", [128, KB * 128], bf16,
                                       isOutput=False)
    out_d = nc.declare_dram_parameter("out", [NOWN, D], f32, isOutput=True)

    with TileContext(nc) as tc:
        with tc.tile_pool(name="const", bufs=1) as const_pool:
            Wh_sb = const_pool.tile([128, KB * DA], bf16)
            nc.sync.dma_start(Wh_sb[:], Wh_d[:])
            wa1r = const_pool.tile([128, KB * 128], bf16)
            nc.sync.dma_start(wa1r[:], wa1r_d[:])
            xparts = [const_pool.tile([128, KB * NQ], bf16, name=f"xp{qt}")
                      for qt in range(4)]
            nc.sync.dma_start(xparts[0][:], xc_d[:, 0:KB * NQ])
            nc.sync.dma_start(xparts[1][:], xc_d[:, KB * NQ:2 * KB * NQ])

            def xkb(k, b):
                """x block [128 k-rows, 128 nodes] for contraction k, node block b."""
                qt, bq = divmod(b, 8)
                off = k * NQ + bq * 128
                return xparts[qt][:, off:off + 128]

            haug = const_pool.tile([128, JB * (D + 1)], bf16)   # [h | 1] per block
            e1all = const_pool.tile([128, JB], f32)             # exp(s2) per block
            gall = const_pool.tile([128, JB], f32)              # exp(0.2 s2) per block
            E8b = const_pool.tile([128, NOWN], bf16)            # exp(-0.8 s1) bcast
            # separate quarter copies of E8b: the first attention blocks run
            # chunked against these so they start before all of E8b is ready
            E8q = [const_pool.tile([128, 512], bf16, name=f"E8q{c}")
                   for c in range(4)]

            # ones columns for the row-sum ride-along, one strided memset
            haug3 = haug.rearrange("p (b c) -> p b c", c=D + 1)
            nc.vector.memset(haug3[:, :, D:D + 1], 1.0)

            with (
                tc.tile_pool(name="stream", bufs=5) as stream,
                tc.tile_pool(name="tail", bufs=1) as tail_pool,
            ):
                def emit_h_block(b, pool):
                    ph = pool.tile([128, D + 1], f32, tag="ph")
                    for k in range(KB):
                        nc.tensor.matmul(
                            ph[:],
                            lhsT=xkb(k, b),
                            rhs=Wh_sb[:, k * DA:k * DA + D + 1],
                            start=(k == 0), stop=(k == KB - 1))
                    nc.scalar.activation(haug[:, b * (D + 1):b * (D + 1) + D],
                                         ph[:, 0:D], Act.Copy)
                    nc.scalar.activation(e1all[:, b:b + 1], ph[:, D:D + 1],
                                         Act.Exp)
                    nc.scalar.activation(gall[:, b:b + 1], ph[:, D:D + 1],
                                         Act.Exp, scale=0.2)

                with tc.tile_pool(name="pre", bufs=2, space="PSUM") as pre:
                    def emit_e8b(c4):
                        # pb[p, i] = s1_i for own cols c4*512.. directly from
                        # x via the replicated-wa1 stationary, then exp
                        pb = pre.tile([128, 512], f32, tag="pb")
                        qt, half = divmod(c4, 2)
                        for k in range(KB):
                            nc.tensor.matmul(
                                pb[:],
                                lhsT=wa1r[:, k * 128:(k + 1) * 128],
                                rhs=xparts[qt][:, k * NQ + half * 512:
                                               k * NQ + (half + 1) * 512],
                                start=(k == 0), stop=(k == KB - 1))
                        nc.scalar.activation(E8q[c4][:], pb[:],
                                             Act.Exp, scale=-0.8)
                        nc.scalar.activation(E8b[:, c4 * 512:(c4 + 1) * 512],
                                             pb[:], Act.Exp, scale=-0.8)

                    # interleave so h-block 0 (whose scores gate the first
                    # attention block) isn't queued behind all E8b matmuls
                    for c4 in range(4):
                        emit_e8b(c4)
                        emit_h_block(c4, pre)

                with (
                    tc.tile_pool(name="acc", bufs=1, space="PSUM") as acc_pool,
                    tc.tile_pool(name="hpsum", bufs=2, space="PSUM") as hpsum,
                ):
                    # 5 tiles x 3 slices + 1 tile x 1 slice = 16 accumulators
                    acc = [acc_pool.tile([128, 3 * (D + 1)], f32,
                                         name=f"acc{t}") for t in range(5)]
                    acc.append(acc_pool.tile([128, D + 1], f32, name="acc5"))

                    masks = {}

                    def emit_mask_dma(jb):
                        mt = stream.tile([128, NOWN], bf16, tag="mask")
                        nc.sync.dma_start(mt[:],
                                          maskT_d[jb * 128:(jb + 1) * 128, :])
                        masks[jb] = mt

                    def emit_attn_block(jb):
                        mask = masks.pop(jb)
                        if jb < 2:
                            # chunked against the E8b quarter tiles so DVE
                            # starts before the whole prelude finishes
                            pmc = []
                            for c in range(4):
                                sl = slice(c * 512, (c + 1) * 512)
                                q1c = stream.tile([128, 512], bf16,
                                                  tag=f"q1c{c}")
                                nc.vector.tensor_scalar(
                                    q1c[:], in0=E8q[c][:],
                                    scalar1=gall[:, jb:jb + 1],
                                    scalar2=e1all[:, jb:jb + 1],
                                    op0=Alu.mult, op1=Alu.max)
                                pc = stream.tile([128, 512], bf16,
                                                 tag=f"pmc{c}")
                                nc.vector.tensor_tensor(pc[:], q1c[:],
                                                        mask[:, sl],
                                                        op=Alu.mult)
                                pmc.append(pc)

                            def pm_slice(ib):
                                c, r = divmod(ib, 4)
                                return pmc[c][:, r * 128:(r + 1) * 128]
                        else:
                            # q1 = max(E8_i * g_j, e1_j) in one two-scalar TS
                            q1 = stream.tile([128, NOWN], bf16, tag="q1")
                            nc.vector.tensor_scalar(q1[:], in0=E8b[:],
                                                    scalar1=gall[:, jb:jb + 1],
                                                    scalar2=e1all[:, jb:jb + 1],
                                                    op0=Alu.mult, op1=Alu.max)
                            pm = stream.tile([128, NOWN], bf16, tag="pm")
                            nc.vector.tensor_tensor(pm[:], q1[:], mask[:],
                                                    op=Alu.mult)

                            def pm_slice(ib):
                                return pm[:, ib * 128:(ib + 1) * 128]

                        for ib in _MM_ORDER:
                            t8, s8 = divmod(ib, 3)
                            last = (s8 == 2) or (t8 == 5)
                            nc.tensor.matmul(
                                acc[t8][:, s8 * (D + 1):(s8 + 1) * (D + 1)],
                                lhsT=pm_slice(ib),
                                rhs=haug[:, jb * (D + 1):(jb + 1) * (D + 1)],
                                start=(jb == 0 and s8 == 0),
                                stop=(jb == JB - 1 and last),
                                skip_group_check=True)

                    for step in range(4, JB + LAG):
                        if step == 6:
                            nc.sync.dma_start(xparts[2][:],
                                              xc_d[:, 2 * KB * NQ:3 * KB * NQ])
                        if step == 12:
                            nc.sync.dma_start(xparts[3][:],
                                              xc_d[:, 3 * KB * NQ:4 * KB * NQ])
                        # masks prefetched two blocks ahead of their use
                        if step == 4:
                            emit_mask_dma(0)
                            emit_mask_dma(1)
                        if step - LAG + 2 < JB:
                            emit_mask_dma(step - LAG + 2)
                        if step < JB:
                            emit_h_block(step, hpsum)
                        if step >= LAG:
                            emit_attn_block(step - LAG)

                    # ---- tail: normalize + elu + store, two fully pipelined
                    # halves (recip -> norm -> elu -> DMA per half) ----
                    osb = tail_pool.tile([128, NOWN], f32, tag="osb")
                    rinv = tail_pool.tile([128, IB], f32, tag="rinv")
                    zmin = tail_pool.tile([128, NOWN], f32, tag="zmin")
                    ez = tail_pool.tile([128, NOWN], f32, tag="ez")
                    rm1 = tail_pool.tile([128, NOWN], f32, tag="rm1")
                    oo = tail_pool.tile([128, NOWN], f32, tag="oo")
                    out3 = out_d.rearrange("(b p) d -> p b d", p=128)
                    oo3 = oo.rearrange("p (b d) -> p b d", d=D)
                    HN = NOWN // 2
                    for hf in range(2):
                        for ib in range(hf * 8, hf * 8 + 8):
                            t8, s = divmod(ib, 3)
                            nc.vector.reciprocal(
                                rinv[:, ib:ib + 1],
                                acc[t8][:, s * (D + 1) + D:
                                        s * (D + 1) + D + 1])
                        for ib in range(hf * 8, hf * 8 + 8):
                            t8, s = divmod(ib, 3)
                            nc.scalar.activation(
                                osb[:, ib * 128:(ib + 1) * 128],
                                acc[t8][:, s * (D + 1):s * (D + 1) + D],
                                Act.Copy, scale=rinv[:, ib:ib + 1])
                        # elu(x) = (relu(x) - 1) + exp(min(x, 0))
                        sl = slice(hf * HN, (hf + 1) * HN)
                        nc.vector.tensor_scalar(zmin[:, sl], in0=osb[:, sl],
                                                scalar1=0.0, scalar2=None,
                                                op0=Alu.min)
                        nc.scalar.activation(ez[:, sl], zmin[:, sl], Act.Exp)
                        nc.vector.tensor_scalar(rm1[:, sl], in0=osb[:, sl],
                                                scalar1=0.0, scalar2=-1.0,
                                                op0=Alu.max, op1=Alu.add)
                        nc.vector.tensor_tensor(oo[:, sl], ez[:, sl],
                                                rm1[:, sl], op=Alu.add)
                        bs = slice(hf * (IB // 2), (hf + 1) * (IB // 2))
                        nc.sync.dma_start(out3[:, bs, :], oo3[:, bs, :])

    nc.compile()
    return nc


def _host_prep(x, adj, W, a):
    import ml_dtypes
    x = np.asarray(x, np.float32)
    adj = np.asarray(adj)
    W = np.asarray(W, np.float32)
    a = np.asarray(a, np.float32)

    xT = x.T.astype(ml_dtypes.bfloat16)                                # [F, N]
    adjT = adj.T.astype(ml_dtypes.bfloat16)                            # [j, i]

    in_maps = []
    for c in range(M):
        hd, q = divmod(c, 2)
        Wh = W[:, hd * D:(hd + 1) * D]                                  # [F, D]
        wa1 = Wh @ a[:D, 0]
        wa2 = Wh @ a[D:, 0]
        # [W_h | wa2 | wa1]: col D = s2 weights, col D+1 = s1 weights;
        # k-interleaved to load with one DMA
        Whc = np.concatenate([Wh, wa2[:, None], wa1[:, None]], axis=1)
        Whc = np.ascontiguousarray(
            Whc.astype(ml_dtypes.bfloat16).reshape(KB, 128, DA)
            .transpose(1, 0, 2).reshape(128, KB * DA))
        # wa1 replicated across 128 stationary columns, k-interleaved
        wa1r = np.ascontiguousarray(
            np.broadcast_to(
                wa1.astype(ml_dtypes.bfloat16).reshape(KB, 128, 1),
                (KB, 128, 128)).transpose(1, 0, 2).reshape(128, KB * 128))
        # rotate the node axis so own nodes are first, then k/quarter-interleave
        xrot = np.concatenate([xT[:, q * NOWN:], xT[:, :q * NOWN]], axis=1)
        xc = np.ascontiguousarray(
            xrot.reshape(KB, 128, 4, NQ).transpose(2, 1, 0, 3).reshape(4, 128, -1)
            .transpose(1, 0, 2).reshape(128, -1))
        adjrot = np.concatenate([adjT[q * NOWN:], adjT[:q * NOWN]], axis=0)
        in_maps.append({
            "xc": xc,
            "Wh": Whc,
            "maskT": np.ascontiguousarray(adjrot[:, q * NOWN:(q + 1) * NOWN]),
            "wa1r": wa1r,
        })
    return in_maps


def kernel(x, adj, W, a):
    from concourse.bass_utils import run_bass_kernel_spmd

    if "nc" not in _CACHE:
        _CACHE["nc"] = _build_nc()
    nc = _CACHE["nc"]

    in_maps = _host_prep(x, adj, W, a)
    res = run_bass_kernel_spmd(nc, in_maps, list(range(M)))
    out = np.empty((N, H * D), np.float32)
    for c in range(M):
        hd, q = divmod(c, 2)
        out[q * NOWN:(q + 1) * NOWN, hd * D:(hd + 1) * D] = np.asarray(
            res.results[c]["out"], np.float32)
    return out


if __name__ == "__main__":
    nc = _build_nc()
    print("built ok")


# revision 62
# speedup vs baseline: 1.0351x; 1.0008x over previous
"""HGATConv (4-head graph attention, N=4096, F=512) on 8 Trainium2 NeuronCores.

Sharding: (head, node-half) grid — core c handles head c//2 and output rows
q*2048..(q+1)*2048 (q = c%2). Every core computes its head's h = x @ W_h for
ALL nodes locally (bf16 matmuls) — no collective at all. The host rotates
the node axis per core so each core's own nodes come first (attention sums
are j-order invariant); that makes the per-core x layout identical SPMD-wise.

Attention math per 128-node j-block (tiles are [128 j, 2048 i]): all scores
are divided by exp(s1_i), which is constant per softmax row and cancels in
the normalization:
  p'' = exp(leakyrelu(s1_i + s2_j)) / exp(s1_i)
      = max(exp(-0.8 s1_i + 0.2 s2_j), exp(s2_j))
      = max(E8_i * g_j, e1_j)          (separable rank-1 first branch!)
with E8_i = exp(-0.8 s1_i) a jb-invariant broadcast tile, g_j = exp(0.2 s2_j)
and e1_j = exp(s2_j) per-partition scalars. So the whole score+leakyrelu is
ONE 4x-rate two-scalar tensor_scalar per block — no wide ACT exp at all; the
only wide 2x op left is the adjacency-mask multiply.
Row sums ride along as a ones-column appended to h (129-wide matmuls).

x arrives in four k-interleaved quarter DMAs so the h-compute starts after
the first MiB; the h-compute loop is emitted interleaved with the attention
loop (lag 4) and PSUM accumulators are packed 3-per-bank so both phases'
PSUM pools coexist and all engines pipeline across the phases.
"""

import sys
import numpy as np

if "/opt/trn_rl_repo" not in sys.path:
    sys.path.insert(0, "/opt/trn_rl_repo")

H, D = 4, 128          # heads, head dim
N, F = 4096, 512       # nodes, features
M = 8                  # cores
NOWN = 1024 * 2        # 2048 own output rows per core
JB = N // 128          # 32 j blocks
IB = NOWN // 128       # 16 own-row blocks
KB = F // 128          # 4 contraction blocks
DA = D + 2             # head W columns + wa2 + wa1
NQ = N // 4            # nodes per x-quarter DMA
LAG = 4                # h-compute blocks emitted ahead of attention blocks

_CACHE = {}

# attention-matmul emission order: s=0 slices first (their start=True clears
# the bank), consecutive matmuls on different PSUM banks
_MM_ORDER = [0, 3, 6, 9, 12, 15, 1, 4, 7, 10, 13, 2, 5, 8, 11, 14]


def _build_nc():
    import concourse.bacc as bacc
    from concourse import mybir
    from concourse.tile import TileContext

    f32 = mybir.dt.float32
    bf16 = mybir.dt.bfloat16
    Alu = mybir.AluOpType
    Act = mybir.ActivationFunctionType

    nc = bacc.Bacc()
    # x, k-interleaved and quarter-major: one DMA slice brings every
    # contraction block for a 1024-node range (device node order is the
    # per-core rotated order; own nodes are quarters 0-1)
    xc_d = nc.declare_dram_parameter("xc", [128, KB * N], bf16, isOutput=False)
    Wh_d = nc.declare_dram_parameter("Wh", [128, KB * DA], bf16, isOutput=False)
    maskT_d = nc.declare_dram_parameter("maskT", [N, NOWN], bf16, isOutput=False)
    # wa1 replicated across 128 columns: E8b comes from one matmul layer
    wa1r_d = nc.declare_dram_parameter("wa1r", [128, KB * 128], bf16,
                                       isOutput=False)
    out_d = nc.declare_dram_parameter("out", [NOWN, D], f32, isOutput=True)

    with TileContext(nc) as tc:
        with tc.tile_pool(name="const", bufs=1) as const_pool:
            Wh_sb = const_pool.tile([128, KB * DA], bf16)
            nc.sync.dma_start(Wh_sb[:], Wh_d[:])
            wa1r = const_pool.tile([128, KB * 128], bf16)
            nc.sync.dma_start(wa1r[:], wa1r_d[:])
            xparts = [const_pool.tile([128, KB * NQ], bf16, name=f"xp{qt}")
                      for qt in range(4)]
            nc.sync.dma_start(xparts[0][:], xc_d[:, 0:KB * NQ])
            nc.sync.dma_start(xparts[1][:], xc_d[:, KB * NQ:2 * KB * NQ])

            def xkb(k, b):
                """x block [128 k-rows, 128 nodes] for contraction k, node block b."""
                qt, bq = divmod(b, 8)
                off = k * NQ + bq * 128
                return xparts[qt][:, off:off + 128]

            haug = const_pool.tile([128, JB * (D + 1)], bf16)   # [h | 1] per block
            e1all = const_pool.tile([128, JB], f32)             # exp(s2) per block
            gall = const_pool.tile([128, JB], f32)              # exp(0.2 s2) per block
            E8b = const_pool.tile([128, NOWN], bf16)            # exp(-0.8 s1) bcast
            # separate quarter copies of E8b: the first attention blocks run
            # chunked against these so they start before all of E8b is ready
            E8q = [const_pool.tile([128, 512], bf16, name=f"E8q{c}")
                   for c in range(4)]

            # ones columns for the row-sum ride-along, one strided memset
            haug3 = haug.rearrange("p (b c) -> p b c", c=D + 1)
            nc.vector.memset(haug3[:, :, D:D + 1], 1.0)

            with (
                tc.tile_pool(name="stream", bufs=5) as stream,
                tc.tile_pool(name="tail", bufs=1) as tail_pool,
            ):
                def emit_h_block(b, pool):
                    ph = pool.tile([128, D + 1], f32, tag="ph")
                    for k in range(KB):
                        nc.tensor.matmul(
                            ph[:],
                            lhsT=xkb(k, b),
                            rhs=Wh_sb[:, k * DA:k * DA + D + 1],
                            start=(k == 0), stop=(k == KB - 1))
                    nc.scalar.activation(haug[:, b * (D + 1):b * (D + 1) + D],
                                         ph[:, 0:D], Act.Copy)
                    nc.scalar.activation(e1all[:, b:b + 1], ph[:, D:D + 1],
                                         Act.Exp)
                    nc.scalar.activation(gall[:, b:b + 1], ph[:, D:D + 1],
                                         Act.Exp, scale=0.2)

                with tc.tile_pool(name="pre", bufs=2, space="PSUM") as pre:
                    def emit_e8b(c4):
                        # pb[p, i] = s1_i for own cols c4*512.. directly from
                        # x via the replicated-wa1 stationary, then exp
                        pb = pre.tile([128, 512], f32, tag="pb")
                        qt, half = divmod(c4, 2)
                        for k in range(KB):
                            nc.tensor.matmul(
                                pb[:],
                                lhsT=wa1r[:, k * 128:(k + 1) * 128],
                                rhs=xparts[qt][:, k * NQ + half * 512:
                                               k * NQ + (half + 1) * 512],
                                start=(k == 0), stop=(k == KB - 1))
                        nc.scalar.activation(E8q[c4][:], pb[:],
                                             Act.Exp, scale=-0.8)
                        nc.scalar.activation(E8b[:, c4 * 512:(c4 + 1) * 512],
                                             pb[:], Act.Exp, scale=-0.8)

                    # interleave so h-block 0 (whose scores gate the first
                    # attention block) isn't queued behind all E8b matmuls
                    for c4 in range(4):
                        emit_e8b(c4)
                        emit_h_block(c4, pre)

                with (
                    tc.tile_pool(name="acc", bufs=1, space="PSUM") as acc_pool,
                    tc.tile_pool(name="hpsum", bufs=2, space="PSUM") as hpsum,
                ):
                    # 5 tiles x 3 slices + 1 tile x 1 slice = 16 accumulators
                    acc = [acc_pool.tile([128, 3 * (D + 1)], f32,
                                         name=f"acc{t}") for t in range(5)]
                    acc.append(acc_pool.tile([128, D + 1], f32, name="acc5"))

                    masks = {}

                    def emit_mask_dma(jb):
                        mt = stream.tile([128, NOWN], bf16, tag="mask")
                        nc.sync.dma_start(mt[:],
                                          maskT_d[jb * 128:(jb + 1) * 128, :])
                        masks[jb] = mt

                    q1s = {}

                    def emit_q1(jb):
                        # q1 = max(E8_i * g_j, e1_j), emitted ahead of the
                        # mask multiply so a ready TS always sits in front of
                        # any mask-waiting TT in the in-order DVE queue
                        if jb < 2:
                            # chunked against the E8b quarter tiles so DVE
                            # starts before the whole prelude finishes
                            qs = []
                            for c in range(4):
                                q1c = stream.tile([128, 512], bf16,
                                                  tag=f"q1c{c}")
                                nc.vector.tensor_scalar(
                                    q1c[:], in0=E8q[c][:],
                                    scalar1=gall[:, jb:jb + 1],
                                    scalar2=e1all[:, jb:jb + 1],
                                    op0=Alu.mult, op1=Alu.max)
                                qs.append(q1c)
                            q1s[jb] = qs
                        else:
                            q1 = stream.tile([128, NOWN], bf16, tag="q1")
                            nc.vector.tensor_scalar(q1[:], in0=E8b[:],
                                                    scalar1=gall[:, jb:jb + 1],
                                                    scalar2=e1all[:, jb:jb + 1],
                                                    op0=Alu.mult, op1=Alu.max)
                            q1s[jb] = q1

                    def emit_attn_block(jb):
                        mask = masks.pop(jb)
                        q = q1s.pop(jb)
                        if jb < 2:
                            pmc = []
                            for c in range(4):
                                sl = slice(c * 512, (c + 1) * 512)
                                pc = stream.tile([128, 512], bf16,
                                                 tag=f"pmc{c}")
                                nc.vector.tensor_tensor(pc[:], q[c][:],
                                                        mask[:, sl],
                                                        op=Alu.mult)
                                pmc.append(pc)

                            def pm_slice(ib):
                                c, r = divmod(ib, 4)
                                return pmc[c][:, r * 128:(r + 1) * 128]
                        else:
                            pm = stream.tile([128, NOWN], bf16, tag="pm")
                            nc.vector.tensor_tensor(pm[:], q[:], mask[:],
                                                    op=Alu.mult)

                            def pm_slice(ib):
                                return pm[:, ib * 128:(ib + 1) * 128]

                        for ib in _MM_ORDER:
                            t8, s8 = divmod(ib, 3)
                            last = (s8 == 2) or (t8 == 5)
                            nc.tensor.matmul(
                                acc[t8][:, s8 * (D + 1):(s8 + 1) * (D + 1)],
                                lhsT=pm_slice(ib),
                                rhs=haug[:, jb * (D + 1):(jb + 1) * (D + 1)],
                                start=(jb == 0 and s8 == 0),
                                stop=(jb == JB - 1 and last),
                                skip_group_check=True)

                    for step in range(4, JB + LAG):
                        if step == 6:
                            nc.sync.dma_start(xparts[2][:],
                                              xc_d[:, 2 * KB * NQ:3 * KB * NQ])
                        if step == 12:
                            nc.sync.dma_start(xparts[3][:],
                                              xc_d[:, 3 * KB * NQ:4 * KB * NQ])
                        # masks and q1 prefetched two blocks ahead of use
                        if step == 4:
                            emit_mask_dma(0)
                            emit_mask_dma(1)
                            emit_q1(0)
                            emit_q1(1)
                        if step - LAG + 2 < JB:
                            emit_mask_dma(step - LAG + 2)
                            emit_q1(step - LAG + 2)
                        if step < JB:
                            emit_h_block(step, hpsum)
                        if step >= LAG:
                            emit_attn_block(step - LAG)

                    # ---- tail: normalize + elu + store, two fully pipelined
                    # halves (recip -> norm -> elu -> DMA per half) ----
                    osb = tail_pool.tile([128, NOWN], f32, tag="osb")
                    rinv = tail_pool.tile([128, IB], f32, tag="rinv")
                    zmin = tail_pool.tile([128, NOWN], f32, tag="zmin")
                    ez = tail_pool.tile([128, NOWN], f32, tag="ez")
                    rm1 = tail_pool.tile([128, NOWN], f32, tag="rm1")
                    oo = tail_pool.tile([128, NOWN], f32, tag="oo")
                    out3 = out_d.rearrange("(b p) d -> p b d", p=128)
                    oo3 = oo.rearrange("p (b d) -> p b d", d=D)
                    HN = NOWN // 2
                    for hf in range(2):
                        for ib in range(hf * 8, hf * 8 + 8):
                            t8, s = divmod(ib, 3)
                            nc.vector.reciprocal(
                                rinv[:, ib:ib + 1],
                                acc[t8][:, s * (D + 1) + D:
                                        s * (D + 1) + D + 1])
                        for ib in range(hf * 8, hf * 8 + 8):
                            t8, s = divmod(ib, 3)
                            nc.scalar.activation(
                                osb[:, ib * 128:(ib + 1) * 128],
                                acc[t8][:, s * (D + 1):s * (D + 1) + D],
                                Act.Copy, scale=rinv[:, ib:ib + 1])
                        # elu(x) = (relu(x) - 1) + exp(min(x, 0))
                        sl = slice(hf * HN, (hf + 1) * HN)
                        nc.vector.tensor_scalar(zmin[:, sl], in0=osb[:, sl],
                                                scalar1=0.0, scalar2=None,
                                                op0=Alu.min)
                        nc.scalar.activation(ez[:, sl], zmin[:, sl], Act.Exp)
                        nc.vector.tensor_scalar(rm1[:, sl], in0=osb[:, sl],
                                                scalar1=0.0, scalar2=-1.0,
                                                op0=Alu.max, op1=Alu.add)
                        nc.vector.tensor_tensor(oo[:, sl], ez[:, sl],
                                                rm1[:, sl], op=Alu.add)
                        bs = slice(hf * (IB // 2), (hf + 1) * (IB // 2))
                        nc.sync.dma_start(out3[:, bs, :], oo3[:, bs, :])

    nc.compile()
    return nc


def _host_prep(x, adj, W, a):
    import ml_dtypes
    x = np.asarray(x, np.float32)
    adj = np.asarray(adj)
    W = np.asarray(W, np.float32)
    a = np.asarray(a, np.float32)

    xT = x.T.astype(ml_dtypes.bfloat16)                                # [F, N]
    adjT = adj.T.astype(ml_dtypes.bfloat16)                            # [j, i]

    in_maps = []
    for c in range(M):
        hd, q = divmod(c, 2)
        Wh = W[:, hd * D:(hd + 1) * D]                                  # [F, D]
        wa1 = Wh @ a[:D, 0]
        wa2 = Wh @ a[D:, 0]
        # [W_h | wa2 | wa1]: col D = s2 weights, col D+1 = s1 weights;
        # k-interleaved to load with one DMA
        Whc = np.concatenate([Wh, wa2[:, None], wa1[:, None]], axis=1)
        Whc = np.ascontiguousarray(
            Whc.astype(ml_dtypes.bfloat16).reshape(KB, 128, DA)
            .transpose(1, 0, 2).reshape(128, KB * DA))
        # wa1 replicated across 128 stationary columns, k-interleaved
        wa1r = np.ascontiguousarray(
            np.broadcast_to(
                wa1.astype(ml_dtypes.bfloat16).reshape(KB, 128, 1),
                (KB, 128, 128)).transpose(1, 0, 2).reshape(128, KB * 128))
        # rotate the node axis so own nodes are first, then k/quarter-interleave
        xrot = np.concatenate([xT[:, q * NOWN:], xT[:, :q * NOWN]], axis=1)
        xc = np.ascontiguousarray(
            xrot.reshape(KB, 128, 4, NQ).transpose(2, 1, 0, 3).reshape(4, 128, -1)
            .transpose(1, 0, 2).reshape(128, -1))
        adjrot = np.concatenate([adjT[q * NOWN:], adjT[:q * NOWN]], axis=0)
        in_maps.append({
            "xc": xc,
            "Wh": Whc,
            "maskT": np.ascontiguousarray(adjrot[:, q * NOWN:(q + 1) * NOWN]),
            "wa1r": wa1r,
        })
    return in_maps


def kernel(x, adj, W, a):
    from concourse.bass_utils import run_bass_kernel_spmd

    if "nc" not in _CACHE:
        _CACHE["nc"] = _build_nc()
    nc = _CACHE["nc"]

    in_maps = _host_prep(x, adj, W, a)
    res = run_bass_kernel_spmd(nc, in_maps, list(range(M)))
    out = np.empty((N, H * D), np.float32)
    for c in range(M):
        hd, q = divmod(c, 2)
        out[q * NOWN:(q + 1) * NOWN, hd * D:(hd + 1) * D] = np.asarray(
            res.results[c]["out"], np.float32)
    return out


if __name__ == "__main__":
    nc = _build_nc()
    print("built ok")
